# revision 33
# baseline (speedup 1.0000x reference)
"""ARMA GNN (nn_Arma) Trainium2 8-core Bass kernel.

Node-sharded (12500/core), K=3 stacks packed to 96-wide (pad 128 bf16).
Per prop step: AllGather u=dinv*z rows into an HBM table; dma_gather in-edge
src rows (token-major [128,slot,128]) rotated across 4 SWDGE queues with
4-deep SBUF buffers; TensorEngine segment-sum psum[dst,f] += S_tile^T @
G_tile with host-baked 0/1 S tiles streamed via HWDGE (f32 PSUM,
race-free); DVE applies out=act(dinv*y+rootb); PE transposes + dense
matmuls produce the next u. 4 prop steps (2 ARMA layers x T=2).
"""
import sys

sys.path.insert(0, "/opt/trn_rl_repo")
import numpy as np
import ml_dtypes

import concourse.bass as bass
import concourse.bacc as bacc
import concourse.mybir as mybir
from concourse._compat import get_trn_type
from concourse.bass_utils import run_bass_kernel_spmd

BF16 = mybir.dt.bfloat16
F32 = mybir.dt.float32
I16 = mybir.dt.int16
ALU = mybir.AluOpType

C = 8
CH = 2048
TPC = CH // 128
BLK = 32768
NQ = 4          # SWDGE queues / gather+S buffer depth
NPS = 4         # segment-sum psum depth
TRACE = [False]
LAST = {}


def _prep_graph(src, dst, N, Nc, Ncp):
    NW = Ncp // 128
    NB = (C * Ncp + BLK - 1) // BLK
    owner = dst // Nc
    urow = (src // Nc) * Ncp + (src % Nc)
    per_core, ngrp = [], NB * NW
    tiles = np.zeros(ngrp, np.int64)
    for c in range(C):
        m = owner == c
        es, ed = urow[m], dst[m] - c * Nc
        key = (ed >> 7) + NW * (es // BLK)
        o = np.lexsort((es, ed, key))
        es, ed, key = es[o], ed[o], key[o]
        per_core.append((es, ed, key))
        cntv = np.bincount(key, minlength=ngrp)
        tiles = np.maximum(tiles, (cntv + 127) // 128)
    sched = []
    for b in range(NB):
        for w in range(NW):
            sched += [(b, w)] * int(tiles[b * NW + w])
        while len(sched) % TPC != 0:
            sched.append((b, NW - 1))
    T = len(sched)
    gidx_all, s_all = [], []
    for c in range(C):
        es, ed, key = per_core[c]
        counts = np.bincount(key, minlength=ngrp)
        starts = np.concatenate([[0], np.cumsum(counts)])
        gtok = np.zeros(T * 128, np.int64)
        S = np.zeros((128, T * 128), np.float32)
        consumed = np.zeros(ngrp, np.int64)
        for i, (b, w) in enumerate(sched):
            g = b * NW + w
            lo = int(consumed[g])
            hi = min(int(counts[g]), lo + 128)
            if hi > lo:
                seg = slice(int(starts[g]) + lo, int(starts[g]) + hi)
                k = hi - lo
                gtok[i * 128:i * 128 + k] = es[seg] - b * BLK
                S[np.arange(k), i * 128 + (ed[seg] - w * 128)] = 1.0
                consumed[g] = hi
        gidx_all.append(np.tile(gtok.reshape(-1, 16).T, (8, 1)).astype(np.int16))
        s_all.append(S.astype(ml_dtypes.bfloat16))
    return sched, gidx_all, s_all


def _build_nc(Nc, Ncp, Fin, sched):
    NW = G = Ncp // 128
    T = len(sched)
    NCH = T // TPC
    TOKC = T * 128 // 16
    NB = (C * Ncp + BLK - 1) // BLK
    UR = NB * BLK
    NCOL = G * 128
    ZG = (NCOL + 511) // 512
    groups = []
    i = 0
    while i < T:
        b, w = sched[i]
        j = i
        # split groups at chunk boundaries so a group never spans chunks:
        # its evac then implies its chunk's buffers are consumed.
        lim = (i // TPC + 1) * TPC
        while j < T and j < lim and sched[j] == (b, w):
            j += 1
        groups.append((b, w, i, j - i))
        i = j
    NGRP_STEP = len(groups)
    grp_of_tile = np.zeros(T, np.int64)
    for gi2, (_b, _w, t0, nt) in enumerate(groups):
        grp_of_tile[t0:t0 + nt] = gi2
    lastgrp_of_chunk = [int(grp_of_tile[(k + 1) * TPC - 1]) for k in range(T // TPC)]

    nc = bacc.Bacc(get_trn_type() or "TRN2", num_swdge_queues=NQ,
                   dynamic_dma_scratch_size=32768)
    x_p = nc.declare_dram_parameter("x_sh", [Ncp, Fin], F32, isOutput=False)
    gidx_p = nc.declare_dram_parameter("gidx", [128, TOKC], I16, isOutput=False)
    s_p = nc.declare_dram_parameter("smat", [128, T * 128], BF16, isOutput=False)
    dinv_p = nc.declare_dram_parameter("dinv", [128, G], F32, isOutput=False)
    wp = {}
    for nm, rows in [("w1i", 64), ("w1r", 64), ("w1b", 128),
                     ("w2i", 32), ("w2r", 32), ("w2b", 128)]:
        wp[nm] = nc.declare_dram_parameter(nm, [rows, 96], BF16, isOutput=False)
    b1_p = nc.declare_dram_parameter("b1c", [128, 1], F32, isOutput=False)
    b2_p = nc.declare_dram_parameter("b2c", [128, 1], F32, isOutput=False)
    id_p = nc.declare_dram_parameter("ident", [128, 128], BF16, isOutput=False)
    out_p = nc.declare_dram_parameter("out", [128, G * 32], F32, isOutput=True)

    agin = nc.dram_tensor("agin", [Ncp, 128], BF16)
    ubuf = nc.dram_tensor("ubuf", [UR, 128], BF16, addr_space="Shared")

    from contextlib import ExitStack
    with ExitStack() as _st, nc.Block() as block:
        gidx_sb = _st.enter_context(nc.sbuf_tensor("gidx_sb", [128, TOKC], I16))
        gbs = [_st.enter_context(nc.sbuf_tensor(f"gb{i}", [128, TPC, 128], BF16))
               for i in range(NQ)]
        sms = [_st.enter_context(nc.sbuf_tensor(f"sm{i}", [128, TPC, 128], BF16))
               for i in range(NQ)]
        yacc = _st.enter_context(nc.sbuf_tensor("yacc", [128, G, 96], BF16))
        onm = _st.enter_context(nc.sbuf_tensor("onm", [128, G, 96], BF16))
        unm = _st.enter_context(nc.sbuf_tensor("unm", [128, G, 128], BF16))
        rbnm = _st.enter_context(nc.sbuf_tensor("rbnm", [128, G, 96], BF16))
        fsA = _st.enter_context(nc.sbuf_tensor("fsA", [128, 512], BF16))
        fsB = _st.enter_context(nc.sbuf_tensor("fsB", [128, 512], BF16))
        zsA = _st.enter_context(nc.sbuf_tensor("zsA", [128, 512], BF16))
        zsB = _st.enter_context(nc.sbuf_tensor("zsB", [128, 512], BF16))
        rsA = _st.enter_context(nc.sbuf_tensor("rsA", [128, 512], BF16))
        rsB = _st.enter_context(nc.sbuf_tensor("rsB", [128, 512], BF16))
        xnm = _st.enter_context(nc.sbuf_tensor("xnm", [128, G, Fin], BF16))
        hnm = _st.enter_context(nc.sbuf_tensor("hnm", [128, G, 32], BF16))
        outf = _st.enter_context(nc.sbuf_tensor("outf", [128, G * 32], F32))
        dv_sb = _st.enter_context(nc.sbuf_tensor("dv_sb", [128, G], F32))
        id_sb = _st.enter_context(nc.sbuf_tensor("id_sb", [128, 128], BF16))
        w1i_sb = _st.enter_context(nc.sbuf_tensor("w1i_sb", [64, 96], BF16))
        w1r_sb = _st.enter_context(nc.sbuf_tensor("w1r_sb", [64, 96], BF16))
        w1b_sb = _st.enter_context(nc.sbuf_tensor("w1b_sb", [128, 96], BF16))
        w2i_sb = _st.enter_context(nc.sbuf_tensor("w2i_sb", [32, 96], BF16))
        w2r_sb = _st.enter_context(nc.sbuf_tensor("w2r_sb", [32, 96], BF16))
        w2b_sb = _st.enter_context(nc.sbuf_tensor("w2b_sb", [128, 96], BF16))
        b1_sb = _st.enter_context(nc.sbuf_tensor("b1_sb", [128, 1], F32))
        b2_sb = _st.enter_context(nc.sbuf_tensor("b2_sb", [128, 1], F32))
        scr = _st.enter_context(nc.sbuf_tensor("scr", [1, 256], BF16))
        dord = _st.enter_context(nc.semaphore("dord"))
        pss = [_st.enter_context(nc.psum_tensor(f"ps{i}", [128, 96], F32))
               for i in range(NPS)]
        ptA = _st.enter_context(nc.psum_tensor("ptA", [128, 128], BF16))
        ptB = _st.enter_context(nc.psum_tensor("ptB", [128, 128], BF16))
        pzA = _st.enter_context(nc.psum_tensor("pzA", [128, 512], F32))
        pzB = _st.enter_context(nc.psum_tensor("pzB", [128, 512], F32))
        sd = _st.enter_context(nc.semaphore("sd"))
        gd = _st.enter_context(nc.semaphore("gd"))
        gq = [_st.enter_context(nc.semaphore(f"gq{i}")) for i in range(NQ)]
        sls = [_st.enter_context(nc.semaphore(f"sl{i}")) for i in range(NQ)]
        pg = _st.enter_context(nc.semaphore("pg"))
        de = _st.enter_context(nc.semaphore("de"))
        pe_s = _st.enter_context(nc.semaphore("pe_s"))
        tv = _st.enter_context(nc.semaphore("tv"))
        dvm = _st.enter_context(nc.semaphore("dvm"))
        acm = _st.enter_context(nc.semaphore("acm"))
        cc = _st.enter_context(nc.semaphore("cc"))
        ud = _st.enter_context(nc.semaphore("ud"))
        ys = _st.enter_context(nc.semaphore("ys"))
        pts = [ptA, ptB]
        pzs = [pzA, pzB]
        prs = pzs  # rm reuses the zm bank after its evac (tv-gated)
        fss, zss, rss = [fsA, fsB], [zsA, zsB], [rsA, rsB]

        _do = [0]

        def stepcfg(t):
            # (src_nm, width, Wz, has_rootb, Wr, bias, relu, final)
            return [
                (xnm, Fin, w1i_sb, True, w1r_sb, b1_sb, True, False),
                (onm, 96, w1b_sb, False, None, None, True, False),
                (hnm, 32, w2i_sb, True, w2r_sb, b2_sb, False, False),
                (onm, 96, w2b_sb, False, None, None, False, True),
            ][t]

        def prep_sched(t):
            has_rb = stepcfg(t)[3]
            ev = []
            for j in range(ZG):
                g0, g1 = j * 4, min(G, j * 4 + 4)
                for g in range(g0, g1):
                    ev.append(("ft", j, g))
                ev.append(("zm", j, None))
                if has_rb:
                    ev.append(("rm", j, None))
                for g in range(g0, g1):
                    ev.append(("bt", j, g))
                if has_rb:
                    for g in range(g0, g1):
                        ev.append(("rt", j, g))
            return ev

        # ============ SYNC ============
        @block.sync
        def _(sy):
            for dsb, prm in [(dv_sb, dinv_p), (id_sb, id_p),
                             (w1i_sb, wp["w1i"]), (w1r_sb, wp["w1r"]),
                             (w1b_sb, wp["w1b"]), (w2i_sb, wp["w2i"]),
                             (w2r_sb, wp["w2r"]), (w2b_sb, wp["w2b"]),
                             (b1_sb, b1_p), (b2_sb, b2_p)]:
                sy.dma_start(dsb[:], prm[:]).then_inc(sd, 16)
            for t in range(4):
                sy.wait_ge(dvm, t + 1)
                sy.dma_start(
                    agin.ap().rearrange("(g p) f -> p g f", p=128), unm[:]
                ).then_inc(ud, 16)
                for k in range(NCH):
                    ci = t * NCH + k
                    if ci >= NQ:
                        tprev, kprev = (ci - NQ) // NCH, (ci - NQ) % NCH
                        sy.wait_ge(de, tprev * NGRP_STEP + lastgrp_of_chunk[kprev] + 1)
                    sy.dma_start(
                        sms[ci % NQ].ap().rearrange("p a b -> p (a b)"),
                        s_p[:, k * CH:(k + 1) * CH],
                    ).then_inc(sls[ci % NQ], 16)
            sy.wait_ge(dvm, 5)
            sy.dma_start(out_p[:], outf[:]).then_inc(ud, 16)

        # ============ GPSIMD ============
        @block.gpsimd
        def _(g: bass.BassGpSimd):
            g.dma_start(gidx_sb[:], gidx_p[:]).then_inc(gd, 16)
            g.dma_start(
                xnm.ap(), x_p.ap().rearrange("(g p) f -> p g f", p=128)
            ).then_inc(gd, 16)
            for t in range(4):
                g.wait_ge(ud, 16 * (t + 1))
                if t > 0:
                    for j in range(NQ):
                        ndone = (t * NCH - j + NQ - 1) // NQ
                        if ndone > 0:
                            g.wait_ge(gq[j], 32 * ndone)
                g.collective_compute(
                    "AllGather", ALU.bypass,
                    replica_groups=[list(range(C))],
                    ins=[agin.ap().opt()], outs=[ubuf[0:C * Ncp, :].opt()],
                ).then_inc(cc, 1)
                g.wait_ge(cc, t + 1)
                for k in range(NCH):
                    ci = t * NCH + k
                    if ci >= NQ:
                        tprev, kprev = (ci - NQ) // NCH, (ci - NQ) % NCH
                        g.wait_ge(de, tprev * NGRP_STEP + lastgrp_of_chunk[kprev] + 1)
                    b = sched[k * TPC][0]
                    for j in range(2):
                        qn = 1 + (2 * ci + j) % 3
                        g.dma_gather(
                            gbs[ci % NQ][:, j * (TPC // 2):(j + 1) * (TPC // 2), :],
                            ubuf[b * BLK:min((b + 1) * BLK, C * Ncp), :],
                            gidx_sb[:, (k * CH + j * (CH // 2)) // 16:(k * CH + (j + 1) * (CH // 2)) // 16],
                            CH // 2, CH // 2, 128, single_packet=False,
                            queue_num=qn,
                        ).then_inc(gq[ci % NQ], 16)

        # ============ TENSOR (PE) ============
        @block.tensor
        def _(te):
            te.wait_ge(sd, 16 * 10)
            te.wait_ge(gd, 32)
            q = 0
            grp = 0
            for t in range(4):
                src, width, wz, has_rb, wr, _, _, _ = stepcfg(t)
                if t > 0:
                    te.wait_ge(acm, t)
                for kind, j, g2 in prep_sched(t):
                    te.wait_ge(pe_s, q)
                    if q >= 2:
                        te.wait_ge(tv, q - 1)
                    if kind == "ft":
                        ins = te.transpose(
                            pts[q % 2][0:width, :], src[:, g2, 0:width], id_sb[:]
                        )
                    elif kind == "zm":
                        te.wait_ge(tv, q)  # fwd evacs of group j done
                        n = min(NCOL, (j + 1) * 512) - j * 512
                        ins = te.matmul(
                            pzs[j % 2][0:96, 0:n], wz[0:width, :],
                            fss[j % 2][0:width, 0:n],
                        )
                    elif kind == "rm":
                        te.wait_ge(tv, q)  # zm evac done; its bank is free
                        n = min(NCOL, (j + 1) * 512) - j * 512
                        ins = te.matmul(
                            prs[j % 2][0:96, 0:n], wr[0:width, :],
                            fss[j % 2][0:width, 0:n],
                        )
                    elif kind == "bt":
                        te.wait_ge(tv, q)  # z evac of group j done
                        s = g2 - j * 4
                        ins = te.transpose(
                            pts[q % 2][:, :],
                            zss[j % 2][:, s * 128:(s + 1) * 128], id_sb[:],
                        )
                    else:  # rt
                        te.wait_ge(tv, q)
                        s = g2 - j * 4
                        ins = te.transpose(
                            pts[q % 2][:, :],
                            rss[j % 2][:, s * 128:(s + 1) * 128], id_sb[:],
                        )
                    ins.then_inc(pe_s, 1)
                    q += 1
                for (b, w, t0, nt) in groups:
                    if grp >= NPS:
                        te.wait_ge(de, grp - (NPS - 1))
                    for jj in range(nt):
                        tile = t0 + jj
                        k, slot = tile // TPC, tile % TPC
                        ci = t * NCH + k
                        if slot == 0 or jj == 0:
                            te.wait_ge(gq[ci % NQ], 32 * (ci // NQ + 1))
                            te.wait_ge(sls[ci % NQ], 16 * (ci // NQ + 1))
                            te.nop()
                        if jj == nt - 1:
                            te.wait_ge(pg, grp)
                            te.nop()
                        ins = te.matmul(
                            pss[grp % NPS][:, :], sms[ci % NQ][:, slot, :],
                            gbs[ci % NQ][:, slot, 0:96],
                            start=(jj == 0), stop=(jj == nt - 1),
                        )
                    ins.then_inc(pg, 1)
                    grp += 1

        # ============ VECTOR (DVE) ============
        @block.vector
        def _(v):
            v.wait_ge(sd, 16 * 10)
            for buf in (zsA, zsB, rsA, rsB):
                v.memset(buf[:], 0.0)
            qc = 0
            grp = 0
            dvv = 0
            for t in range(4):
                _, width, _, has_rb, _, bia, relu, final = stepcfg(t)
                for kind, j, g2 in prep_sched(t):
                    v.wait_ge(tv, qc)
                    v.wait_ge(pe_s, qc + 1)
                    if kind == "ft":
                        s = g2 - j * 4
                        ins = v.tensor_copy(
                            fss[j % 2][0:width, s * 128:(s + 1) * 128],
                            pts[qc % 2][0:width, :],
                        )
                    elif kind == "zm":
                        n = min(NCOL, (j + 1) * 512) - j * 512
                        ins = v.tensor_copy(
                            zss[j % 2][0:96, 0:n], pzs[j % 2][0:96, 0:n]
                        )
                    elif kind == "rm":
                        n = min(NCOL, (j + 1) * 512) - j * 512
                        ins = v.tensor_scalar_add(
                            rss[j % 2][0:96, 0:n], prs[j % 2][0:96, 0:n],
                            bia[0:96, 0:1],
                        )
                    elif kind == "bt":
                        ins = v.tensor_scalar_mul(
                            unm[:, g2, :], pts[qc % 2][:, :], dv_sb[:, g2:g2 + 1]
                        )
                    else:  # rt
                        ins = v.tensor_copy(rbnm[:, g2, :], pts[qc % 2][:, 0:96])
                    ins.then_inc(tv, 1)
                    qc += 1
                dvv += 1
                v.wait_ge(tv, qc)
                v.wait_ge(dvm, dvv - 1)
                v.nop().then_inc(dvm, 1)  # u_t ready
                # EXEC: accumulate segsum groups
                v.wait_ge(ys, t)
                v.memset(yacc[:], 0.0).then_inc(ys, 1)
                v.wait_ge(ys, t + 1)
                for gi in range(NGRP_STEP):
                    w = groups[gi][1]
                    v.wait_ge(de, grp)
                    v.wait_ge(pg, grp + 1)
                    v.tensor_add(
                        yacc[:, w, :], pss[grp % NPS][:, :], yacc[:, w, :]
                    ).then_inc(de, 1)
                    grp += 1
                # activation: out = act(dinv*y + rootb)
                v.wait_ge(de, grp)
                v.memset(scr[0:1, 0:1], 0.0)
                if not final:
                    v.wait_ge(acm, t)
                    v.memset(scr[0:1, 1:2], 0.0)
                def dsync(ins):
                    _do[0] += 1
                    ins.then_inc(dord, 1)
                    v.wait_ge(dord, _do[0])
                    v.memset(scr[0:1, _do[0] % 256:_do[0] % 256 + 1], 0.0)

                last = None
                for w in range(G):
                    last = v.scalar_tensor_tensor(
                        onm[:, w, :], yacc[:, w, :], dv_sb[:, w:w + 1],
                        rbnm[:, w, :], ALU.mult, ALU.add,
                    )
                if relu or final:
                    dsync(last)
                if relu:
                    last = v.tensor_scalar_max(onm[:], onm[:], 0.0)
                    if t == 1:
                        dsync(last)
                if t == 1:
                    i1 = v.tensor_add(hnm[:], onm[:, :, 0:32], onm[:, :, 32:64])
                    dsync(i1)
                    last = v.tensor_add(hnm[:], hnm[:], onm[:, :, 64:96])
                if final:
                    o3 = outf.ap().rearrange("p (g f) -> p g f", f=32)
                    i1 = v.tensor_add(o3, onm[:, :, 0:32], onm[:, :, 32:64])
                    dsync(i1)
                    i2 = v.tensor_add(o3, o3, onm[:, :, 64:96])
                    dsync(i2)
                    v.wait_ge(dvm, 4)
                    v.tensor_scalar_mul(o3, o3, 1.0 / 3.0).then_inc(dvm, 1)
                else:
                    last.then_inc(acm, 1)

    nc.compile()
    return nc


def _pack_weights(inputs):
    f = lambda a: np.asarray(a, np.float32)
    w1i = f(inputs["w1_init"])          # [3,64,32]
    w1 = f(inputs["w1"])[0]             # [3,32,32]
    w1r = f(inputs["w1_root"])[0]       # [3,64,32]
    b1 = f(inputs["b1"])[0]             # [3,1,32]
    w2i = f(inputs["w2_init"])          # [3,32,32]
    w2 = f(inputs["w2"])[0]
    w2r = f(inputs["w2_root"])[0]
    b2 = f(inputs["b2"])[0]
    cat = lambda w: np.concatenate([w[k] for k in range(3)], axis=1)  # [in,96]
    blk = lambda w: np.block([
        [w[0], np.zeros((32, 64), np.float32)],
        [np.zeros((32, 32), np.float32), w[1], np.zeros((32, 32), np.float32)],
        [np.zeros((32, 64), np.float32), w[2]],
    ])  # [96,96]
    pad128 = lambda a: np.concatenate(
        [a, np.zeros((128 - a.shape[0], a.shape[1]), np.float32)], axis=0)
    bf = lambda a: np.ascontiguousarray(a).astype(ml_dtypes.bfloat16)
    out = {
        "w1i": bf(cat(w1i)),
        "w1r": bf(cat(w1r)),
        "w1b": bf(pad128(blk(w1))),
        "w2i": bf(cat(w2i) / 3.0),      # fold mean-over-stacks of h
        "w2r": bf(cat(w2r) / 3.0),
        "w2b": bf(pad128(blk(w2))),
        "b1c": np.ascontiguousarray(np.concatenate([b1[:, 0, :].reshape(-1), np.zeros(32, np.float32)]).reshape(128, 1)),
        "b2c": np.ascontiguousarray(np.concatenate([b2[:, 0, :].reshape(-1), np.zeros(32, np.float32)]).reshape(128, 1)),
        "ident": bf(np.eye(128, dtype=np.float32)),
    }
    return out


def kernel(**inputs):
    x = np.asarray(inputs["x"], np.float32)
    ei = np.asarray(inputs["edge_index"], np.int64)
    N, Fin = x.shape
    Nc = N // C
    Ncp = ((Nc + 127) // 128) * 128
    src, dst = ei[0], ei[1]

    deg = np.bincount(dst, minlength=N).astype(np.float32)
    dinv = np.where(deg > 0, 1.0 / np.sqrt(np.maximum(deg, 1.0)), 0.0).astype(np.float32)

    sched, gidx_all, s_all = _prep_graph(src, dst, N, Nc, Ncp)
    wts = _pack_weights(inputs)
    nc = _build_nc(Nc, Ncp, Fin, sched)

    G = Ncp // 128
    in_maps = []
    for c in range(C):
        xs = np.zeros((Ncp, Fin), np.float32)
        xs[:Nc] = x[c * Nc:(c + 1) * Nc]
        dl = np.zeros(Ncp, np.float32)
        dl[:Nc] = dinv[c * Nc:(c + 1) * Nc]
        dnm = dl.reshape(G, 128).T.copy()          # [128, G]
        m = dict(wts)
        m.update(
            x_sh=xs, gidx=gidx_all[c], smat=s_all[c],
            dinv=np.ascontiguousarray(dnm),
        )
        in_maps.append(m)

    res = run_bass_kernel_spmd(nc, in_maps, list(range(C)), trace=TRACE[0])
    LAST["res"] = res

    out = np.zeros((N, 32), np.float32)
    for c in range(C):
        oc = np.asarray(res.results[c]["out"], np.float32)  # [128, G*32]
        oc = oc.reshape(128, G, 32).transpose(1, 0, 2).reshape(Ncp, 32)
        out[c * Nc:(c + 1) * Nc] = oc[:Nc]
    return out


# revision 58
# speedup vs baseline: 1.0929x; 1.0929x over previous
"""ARMA GNN (nn_Arma) Trainium2 8-core Bass kernel.

Node-sharded (12500/core), K=3 stacks packed to 96-wide (pad 128 bf16).
Per prop step: AllGather u=dinv*z rows into an HBM table; dma_gather in-edge
src rows (token-major [128,slot,128]) on SWDGE queues 1-3 with 4-deep SBUF
buffers; TensorEngine segment-sum psum[dst,f] += S_tile^T @ G_tile with 0/1
S tiles built ON-CHIP by one DVE broadcast is_equal op per chunk (f32 PSUM,
race-free); DVE applies out=act(dinv*y+rootb); PE transposes + dense
matmuls produce the next u. 4 prop steps (2 ARMA layers x T=2).
"""
import sys

sys.path.insert(0, "/opt/trn_rl_repo")
import numpy as np
import ml_dtypes

import concourse.bass as bass
import concourse.bacc as bacc
import concourse.mybir as mybir
from concourse._compat import get_trn_type
from concourse.bass_utils import run_bass_kernel_spmd

BF16 = mybir.dt.bfloat16
F32 = mybir.dt.float32
I16 = mybir.dt.int16
ALU = mybir.AluOpType

C = 8
CH = 2048
TPC = CH // 128
BLK = 32768
NQ = 4          # SWDGE queues / gather+S buffer depth
NPS = 4         # segment-sum psum depth
TRACE = [False]
LAST = {}


def _prep_graph(src, dst, N, Nc, Ncp):
    NW = Ncp // 128
    NB = (C * Ncp + BLK - 1) // BLK
    owner = dst // Nc
    urow = (src // Nc) * Ncp + (src % Nc)
    per_core, ngrp = [], NB * NW
    tiles = np.zeros(ngrp, np.int64)
    for c in range(C):
        m = owner == c
        es, ed = urow[m], dst[m] - c * Nc
        key = (ed >> 7) + NW * (es // BLK)
        o = np.lexsort((es, ed, key))
        es, ed, key = es[o], ed[o], key[o]
        per_core.append((es, ed, key))
        cntv = np.bincount(key, minlength=ngrp)
        tiles = np.maximum(tiles, (cntv + 127) // 128)
    sched = []
    for b in range(NB):
        for w in range(NW):
            sched += [(b, w)] * int(tiles[b * NW + w])
        while len(sched) % TPC != 0:
            sched.append((b, NW - 1))
    T = len(sched)
    gidx_all, dloc_all = [], []
    for c in range(C):
        es, ed, key = per_core[c]
        counts = np.bincount(key, minlength=ngrp)
        starts = np.concatenate([[0], np.cumsum(counts)])
        gtok = np.zeros(T * 128, np.int64)
        dloc = np.full(T * 128, -1.0, np.float32)
        consumed = np.zeros(ngrp, np.int64)
        for i, (b, w) in enumerate(sched):
            g = b * NW + w
            lo = int(consumed[g])
            hi = min(int(counts[g]), lo + 128)
            if hi > lo:
                seg = slice(int(starts[g]) + lo, int(starts[g]) + hi)
                k = hi - lo
                gtok[i * 128:i * 128 + k] = es[seg] - b * BLK
                dloc[i * 128:i * 128 + k] = (ed[seg] - w * 128).astype(np.float32)
                consumed[g] = hi
        gidx_all.append(np.tile(gtok.reshape(-1, 16).T, (8, 1)).astype(np.int16))
        dloc_all.append(np.ascontiguousarray(
            dloc.reshape(T, 128).T).astype(ml_dtypes.bfloat16))
    return sched, gidx_all, dloc_all


def _build_nc(Nc, Ncp, Fin, sched):
    NW = G = Ncp // 128
    T = len(sched)
    NCH = T // TPC
    TOKC = T * 128 // 16
    NB = (C * Ncp + BLK - 1) // BLK
    UR = NB * BLK
    NCOL = G * 128
    ZG = (NCOL + 511) // 512
    groups = []
    i = 0
    while i < T:
        b, w = sched[i]
        j = i
        # split groups at chunk boundaries so a group never spans chunks:
        # its evac then implies its chunk's buffers are consumed.
        lim = (i // TPC + 1) * TPC
        while j < T and j < lim and sched[j] == (b, w):
            j += 1
        groups.append((b, w, i, j - i))
        i = j
    NGRP_STEP = len(groups)
    grp_of_tile = np.zeros(T, np.int64)
    for gi2, (_b, _w, t0, nt) in enumerate(groups):
        grp_of_tile[t0:t0 + nt] = gi2
    lastgrp_of_chunk = [int(grp_of_tile[(k + 1) * TPC - 1]) for k in range(T // TPC)]

    nc = bacc.Bacc(get_trn_type() or "TRN2", num_swdge_queues=NQ,
                   dynamic_dma_scratch_size=24576)
    x_p = nc.declare_dram_parameter("x_sh", [Ncp, Fin], F32, isOutput=False)
    gidx_p = nc.declare_dram_parameter("gidx", [128, TOKC], I16, isOutput=False)
    dloc_p = nc.declare_dram_parameter("dloc", [128, T], BF16, isOutput=False)
    iota_p = nc.declare_dram_parameter("iotac", [128, 128], BF16, isOutput=False)
    dinv_p = nc.declare_dram_parameter("dinv", [128, G], F32, isOutput=False)
    wp = {}
    for nm, rows in [("w1i", 64), ("w1r", 64), ("w1b", 128),
                     ("w2i", 32), ("w2r", 32), ("w2b", 128)]:
        wp[nm] = nc.declare_dram_parameter(nm, [rows, 96], BF16, isOutput=False)
    b1_p = nc.declare_dram_parameter("b1c", [128, 1], F32, isOutput=False)
    b2_p = nc.declare_dram_parameter("b2c", [128, 1], F32, isOutput=False)
    id_p = nc.declare_dram_parameter("ident", [128, 128], BF16, isOutput=False)
    out_p = nc.declare_dram_parameter("out", [128, G * 32], F32, isOutput=True)

    agin = nc.dram_tensor("agin", [Ncp, 128], BF16)
    ubuf = nc.dram_tensor("ubuf", [UR, 128], BF16, addr_space="Shared")

    from contextlib import ExitStack
    with ExitStack() as _st, nc.Block() as block:
        gidx_sb = _st.enter_context(nc.sbuf_tensor("gidx_sb", [128, TOKC], I16))
        dloc_sb = _st.enter_context(nc.sbuf_tensor("dloc_sb", [128, T], BF16))
        iota_sb = _st.enter_context(nc.sbuf_tensor("iota_sb", [128, 128], BF16))
        gbs = [_st.enter_context(nc.sbuf_tensor(f"gb{i}", [128, TPC, 128], BF16))
               for i in range(NQ)]
        sms = [_st.enter_context(nc.sbuf_tensor(f"sm{i}", [128, TPC, 128], BF16))
               for i in range(NQ)]
        yacc = _st.enter_context(nc.sbuf_tensor("yacc", [128, G, 96], BF16))
        onm = _st.enter_context(nc.sbuf_tensor("onm", [128, G, 96], BF16))
        unm = _st.enter_context(nc.sbuf_tensor("unm", [128, G, 128], BF16))
        rbnm = _st.enter_context(nc.sbuf_tensor("rbnm", [128, G, 96], BF16))
        fsA = _st.enter_context(nc.sbuf_tensor("fsA", [128, 512], BF16))
        fsB = _st.enter_context(nc.sbuf_tensor("fsB", [128, 512], BF16))
        zsA = _st.enter_context(nc.sbuf_tensor("zsA", [128, 512], BF16))
        zsB = _st.enter_context(nc.sbuf_tensor("zsB", [128, 512], BF16))
        rsA = _st.enter_context(nc.sbuf_tensor("rsA", [128, 512], BF16))
        rsB = _st.enter_context(nc.sbuf_tensor("rsB", [128, 512], BF16))
        xnm = _st.enter_context(nc.sbuf_tensor("xnm", [128, G, Fin], BF16))
        hnm = _st.enter_context(nc.sbuf_tensor("hnm", [128, G, 32], BF16))
        # outf aliases xnm: x is consumed in step-0 prep, outf written in step 3
        outf = xnm.ap().rearrange("p g f -> p (g f)").bitcast(F32)
        dv_sb = _st.enter_context(nc.sbuf_tensor("dv_sb", [128, G], F32))
        id_sb = _st.enter_context(nc.sbuf_tensor("id_sb", [128, 128], BF16))
        w1i_sb = _st.enter_context(nc.sbuf_tensor("w1i_sb", [64, 96], BF16))
        w1r_sb = _st.enter_context(nc.sbuf_tensor("w1r_sb", [64, 96], BF16))
        w1b_sb = _st.enter_context(nc.sbuf_tensor("w1b_sb", [128, 96], BF16))
        w2i_sb = _st.enter_context(nc.sbuf_tensor("w2i_sb", [32, 96], BF16))
        w2r_sb = _st.enter_context(nc.sbuf_tensor("w2r_sb", [32, 96], BF16))
        w2b_sb = _st.enter_context(nc.sbuf_tensor("w2b_sb", [128, 96], BF16))
        b1_sb = _st.enter_context(nc.sbuf_tensor("b1_sb", [128, 1], F32))
        b2_sb = _st.enter_context(nc.sbuf_tensor("b2_sb", [128, 1], F32))
        scr = _st.enter_context(nc.sbuf_tensor("scr", [1, 256], BF16))
        dord = _st.enter_context(nc.semaphore("dord"))
        psb = [_st.enter_context(nc.psum_tensor(f"ps{i}", [128, 96], F32))
               for i in range(NPS)]

        def pslice(g):
            return psb[g % NPS][:, :]
        ptA = _st.enter_context(nc.psum_tensor("ptA", [128, 128], BF16))
        ptB = _st.enter_context(nc.psum_tensor("ptB", [128, 128], BF16))
        pzA = _st.enter_context(nc.psum_tensor("pzA", [128, 512], F32))
        pzB = _st.enter_context(nc.psum_tensor("pzB", [128, 512], F32))
        sd = _st.enter_context(nc.semaphore("sd"))
        gd = _st.enter_context(nc.semaphore("gd"))
        gq = [_st.enter_context(nc.semaphore(f"gq{i}")) for i in range(NQ)]
        sbd = _st.enter_context(nc.semaphore("sbd"))
        pg = _st.enter_context(nc.semaphore("pg"))
        de = _st.enter_context(nc.semaphore("de"))
        pe_s = _st.enter_context(nc.semaphore("pe_s"))
        tv = _st.enter_context(nc.semaphore("tv"))
        dvm = _st.enter_context(nc.semaphore("dvm"))
        acm = _st.enter_context(nc.semaphore("acm"))
        cc = _st.enter_context(nc.semaphore("cc"))
        ud = _st.enter_context(nc.semaphore("ud"))
        ys = _st.enter_context(nc.semaphore("ys"))
        pts = [ptA, ptB]
        pzs = [pzA, pzB]
        prs = pzs  # rm reuses the zm bank after its evac (tv-gated)
        fss, zss, rss = [fsA, fsB], [zsA, zsB], [rsA, rsB]

        _do = [0]

        def stepcfg(t):
            # (src_nm, width, Wz, has_rootb, Wr, bias, relu, final)
            return [
                (xnm, Fin, w1i_sb, True, w1r_sb, b1_sb, True, False),
                (onm, 96, w1b_sb, False, None, None, True, False),
                (hnm, 32, w2i_sb, True, w2r_sb, b2_sb, False, False),
                (onm, 96, w2b_sb, False, None, None, False, True),
            ][t]

        def prep_sched(t):
            has_rb = stepcfg(t)[3]
            ev = []
            for j in range(ZG):
                g0, g1 = j * 4, min(G, j * 4 + 4)
                for g in range(g0, g1):
                    ev.append(("ft", j, g))
                ev.append(("zm", j, None))
                if has_rb:
                    ev.append(("rm", j, None))
                for g in range(g0, g1):
                    ev.append(("bt", j, g))
                if has_rb:
                    for g in range(g0, g1):
                        ev.append(("rt", j, g))
            return ev

        # ============ SYNC ============
        @block.sync
        def _(sy):
            for dsb, prm in [(dv_sb, dinv_p), (id_sb, id_p),
                             (dloc_sb, dloc_p), (iota_sb, iota_p),
                             (w1i_sb, wp["w1i"]), (w1r_sb, wp["w1r"]),
                             (w1b_sb, wp["w1b"]), (w2i_sb, wp["w2i"]),
                             (w2r_sb, wp["w2r"]), (w2b_sb, wp["w2b"]),
                             (b1_sb, b1_p), (b2_sb, b2_p)]:
                sy.dma_start(dsb[:], prm[:]).then_inc(sd, 16)
            for t in range(4):
                sy.wait_ge(dvm, t + 1)
                sy.dma_start(
                    agin.ap().rearrange("(g p) f -> p g f", p=128), unm[:]
                ).then_inc(ud, 16)
            sy.wait_ge(dvm, 5)
            sy.dma_start(out_p[:], outf).then_inc(ud, 16)

        # ============ GPSIMD ============
        @block.gpsimd
        def _(g: bass.BassGpSimd):
            g.dma_start(gidx_sb[:], gidx_p[:]).then_inc(gd, 16)
            g.dma_start(
                xnm.ap(), x_p.ap().rearrange("(g p) f -> p g f", p=128)
            ).then_inc(gd, 16)
            for t in range(4):
                g.wait_ge(ud, 16 * (t + 1))
                if t > 0:
                    for j in range(NQ):
                        ndone = (t * NCH - j + NQ - 1) // NQ
                        if ndone > 0:
                            g.wait_ge(gq[j], 16 * ndone)
                g.collective_compute(
                    "AllGather", ALU.bypass,
                    replica_groups=[list(range(C))],
                    ins=[agin.ap().opt()], outs=[ubuf[0:C * Ncp, :].opt()],
                ).then_inc(cc, 1)
                g.wait_ge(cc, t + 1)
                for k in range(NCH):
                    ci = t * NCH + k
                    if ci >= NQ:
                        tprev, kprev = (ci - NQ) // NCH, (ci - NQ) % NCH
                        g.wait_ge(de, tprev * NGRP_STEP + lastgrp_of_chunk[kprev] + 1)
                    b = sched[k * TPC][0]
                    g.dma_gather(
                        gbs[ci % NQ][:], ubuf[b * BLK:min((b + 1) * BLK, C * Ncp), :],
                        gidx_sb[:, k * (CH // 16):(k + 1) * (CH // 16)],
                        CH, CH, 128, single_packet=False,
                        queue_num=1 + ci % 3,
                    ).then_inc(gq[ci % NQ], 16)

        # ============ TENSOR (PE) ============
        @block.tensor
        def _(te):
            te.wait_ge(sd, 16 * 12)
            te.wait_ge(gd, 32)
            q = 0
            grp = 0
            for t in range(4):
                src, width, wz, has_rb, wr, _, _, _ = stepcfg(t)
                if t > 0:
                    te.wait_ge(acm, t)
                for kind, j, g2 in prep_sched(t):
                    te.wait_ge(pe_s, q)
                    if q >= 2:
                        te.wait_ge(tv, q - 1)
                    if kind == "ft":
                        ins = te.transpose(
                            pts[q % 2][0:width, :], src[:, g2, 0:width], id_sb[:]
                        )
                    elif kind == "zm":
                        te.wait_ge(tv, q)  # fwd evacs of group j done
                        n = min(NCOL, (j + 1) * 512) - j * 512
                        ins = te.matmul(
                            pzs[j % 2][0:96, 0:n], wz[0:width, :],
                            fss[j % 2][0:width, 0:n],
                        )
                    elif kind == "rm":
                        te.wait_ge(tv, q)  # zm evac done; its bank is free
                        n = min(NCOL, (j + 1) * 512) - j * 512
                        ins = te.matmul(
                            prs[j % 2][0:96, 0:n], wr[0:width, :],
                            fss[j % 2][0:width, 0:n],
                        )
                    elif kind == "bt":
                        te.wait_ge(tv, q)  # z evac of group j done
                        s = g2 - j * 4
                        ins = te.transpose(
                            pts[q % 2][:, :],
                            zss[j % 2][:, s * 128:(s + 1) * 128], id_sb[:],
                        )
                    else:  # rt
                        te.wait_ge(tv, q)
                        s = g2 - j * 4
                        ins = te.transpose(
                            pts[q % 2][:, :],
                            rss[j % 2][:, s * 128:(s + 1) * 128], id_sb[:],
                        )
                    ins.then_inc(pe_s, 1)
                    q += 1
                for (b, w, t0, nt) in groups:
                    if grp >= NPS:
                        te.wait_ge(de, grp - (NPS - 1))
                    for jj in range(nt):
                        tile = t0 + jj
                        k, slot = tile // TPC, tile % TPC
                        ci = t * NCH + k
                        if slot == 0 or jj == 0:
                            te.wait_ge(gq[ci % NQ], 16 * (ci // NQ + 1))
                            te.wait_ge(sbd, ci + 1)
                            te.nop()
                        if jj == nt - 1:
                            te.wait_ge(pg, grp)
                            te.nop()
                        ins = te.matmul(
                            pslice(grp), sms[ci % NQ][:, slot, :],
                            gbs[ci % NQ][:, slot, 0:96],
                            start=(jj == 0), stop=(jj == nt - 1),
                        )
                    ins.then_inc(pg, 1)
                    grp += 1

        # ============ VECTOR (DVE) ============
        @block.vector
        def _(v):
            v.wait_ge(sd, 16 * 12)
            for buf in (zsA, zsB, rsA, rsB):
                v.memset(buf[:], 0.0)

            def build_s(gci):
                # one-hot S tiles for the whole chunk in one broadcast op
                # (same dloc every step: dloc depends only on k = gci % NCH)
                k = gci % NCH
                iota_bc = iota_sb.ap().unsqueeze(1).broadcast_to([128, TPC, 128])
                dloc_bc = (dloc_sb.ap()[:, k * TPC:(k + 1) * TPC]
                           .unsqueeze(2).broadcast_to([128, TPC, 128]))
                v.tensor_tensor(
                    sms[gci % NQ][:, :, :], iota_bc, dloc_bc, ALU.is_equal
                ).then_inc(sbd, 1)

            for gci in range(NQ):
                build_s(gci)
            qc = 0
            grp = 0
            dvv = 0
            for t in range(4):
                _, width, _, has_rb, _, bia, relu, final = stepcfg(t)
                for kind, j, g2 in prep_sched(t):
                    v.wait_ge(tv, qc)
                    v.wait_ge(pe_s, qc + 1)
                    if kind == "ft":
                        s = g2 - j * 4
                        ins = v.tensor_copy(
                            fss[j % 2][0:width, s * 128:(s + 1) * 128],
                            pts[qc % 2][0:width, :],
                        )
                    elif kind == "zm":
                        n = min(NCOL, (j + 1) * 512) - j * 512
                        ins = v.tensor_copy(
                            zss[j % 2][0:96, 0:n], pzs[j % 2][0:96, 0:n]
                        )
                    elif kind == "rm":
                        n = min(NCOL, (j + 1) * 512) - j * 512
                        ins = v.tensor_scalar_add(
                            rss[j % 2][0:96, 0:n], prs[j % 2][0:96, 0:n],
                            bia[0:96, 0:1],
                        )
                    elif kind == "bt":
                        ins = v.tensor_scalar_mul(
                            unm[:, g2, :], pts[qc % 2][:, :], dv_sb[:, g2:g2 + 1]
                        )
                    else:  # rt
                        ins = v.tensor_copy(rbnm[:, g2, :], pts[qc % 2][:, 0:96])
                    ins.then_inc(tv, 1)
                    qc += 1
                dvv += 1
                v.wait_ge(tv, qc)
                v.wait_ge(dvm, dvv - 1)
                v.nop().then_inc(dvm, 1)  # u_t ready
                # EXEC: accumulate segsum groups
                v.wait_ge(ys, t)
                v.memset(yacc[:], 0.0).then_inc(ys, 1)
                v.wait_ge(ys, t + 1)
                for gi in range(NGRP_STEP):
                    w = groups[gi][1]
                    k_last = (groups[gi][2] + groups[gi][3] - 1) // TPC
                    v.wait_ge(de, grp)
                    v.wait_ge(pg, grp + 1)
                    v.tensor_add(
                        yacc[:, w, :], pslice(grp), yacc[:, w, :]
                    ).then_inc(de, 1)
                    grp += 1
                    # refill S for chunk gci+NQ once its last group is evac'd
                    if gi == NGRP_STEP - 1 or groups[gi + 1][2] // TPC != k_last:
                        gci = t * NCH + k_last
                        if gci + NQ < 4 * NCH:
                            build_s(gci + NQ)
                # activation: out = act(dinv*y + rootb)
                v.wait_ge(de, grp)
                v.memset(scr[0:1, 0:1], 0.0)
                if not final:
                    v.wait_ge(acm, t)
                    v.memset(scr[0:1, 1:2], 0.0)
                def dsync(ins):
                    _do[0] += 1
                    ins.then_inc(dord, 1)
                    v.wait_ge(dord, _do[0])
                    v.memset(scr[0:1, _do[0] % 256:_do[0] % 256 + 1], 0.0)

                last = None
                for w in range(G):
                    last = v.scalar_tensor_tensor(
                        onm[:, w, :], yacc[:, w, :], dv_sb[:, w:w + 1],
                        rbnm[:, w, :], ALU.mult, ALU.add,
                    )
                if relu or final:
                    dsync(last)
                if relu:
                    last = v.tensor_scalar_max(onm[:], onm[:], 0.0)
                    if t == 1:
                        dsync(last)
                if t == 1:
                    i1 = v.tensor_add(hnm[:], onm[:, :, 0:32], onm[:, :, 32:64])
                    dsync(i1)
                    last = v.tensor_add(hnm[:], hnm[:], onm[:, :, 64:96])
                if final:
                    o3 = outf.rearrange("p (g f) -> p g f", f=32)
                    i1 = v.tensor_add(o3, onm[:, :, 0:32], onm[:, :, 32:64])
                    dsync(i1)
                    i2 = v.tensor_add(o3, o3, onm[:, :, 64:96])
                    dsync(i2)
                    v.wait_ge(dvm, 4)
                    v.tensor_scalar_mul(o3, o3, 1.0 / 3.0).then_inc(dvm, 1)
                else:
                    last.then_inc(acm, 1)

    nc.compile()
    return nc


def _pack_weights(inputs):
    f = lambda a: np.asarray(a, np.float32)
    w1i = f(inputs["w1_init"])          # [3,64,32]
    w1 = f(inputs["w1"])[0]             # [3,32,32]
    w1r = f(inputs["w1_root"])[0]       # [3,64,32]
    b1 = f(inputs["b1"])[0]             # [3,1,32]
    w2i = f(inputs["w2_init"])          # [3,32,32]
    w2 = f(inputs["w2"])[0]
    w2r = f(inputs["w2_root"])[0]
    b2 = f(inputs["b2"])[0]
    cat = lambda w: np.concatenate([w[k] for k in range(3)], axis=1)  # [in,96]
    blk = lambda w: np.block([
        [w[0], np.zeros((32, 64), np.float32)],
        [np.zeros((32, 32), np.float32), w[1], np.zeros((32, 32), np.float32)],
        [np.zeros((32, 64), np.float32), w[2]],
    ])  # [96,96]
    pad128 = lambda a: np.concatenate(
        [a, np.zeros((128 - a.shape[0], a.shape[1]), np.float32)], axis=0)
    bf = lambda a: np.ascontiguousarray(a).astype(ml_dtypes.bfloat16)
    out = {
        "w1i": bf(cat(w1i)),
        "w1r": bf(cat(w1r)),
        "w1b": bf(pad128(blk(w1))),
        "w2i": bf(cat(w2i) / 3.0),      # fold mean-over-stacks of h
        "w2r": bf(cat(w2r) / 3.0),
        "w2b": bf(pad128(blk(w2))),
        "b1c": np.ascontiguousarray(np.concatenate([b1[:, 0, :].reshape(-1), np.zeros(32, np.float32)]).reshape(128, 1)),
        "b2c": np.ascontiguousarray(np.concatenate([b2[:, 0, :].reshape(-1), np.zeros(32, np.float32)]).reshape(128, 1)),
        "ident": bf(np.eye(128, dtype=np.float32)),
        "iotac": bf(np.tile(np.arange(128, dtype=np.float32), (128, 1))),
    }
    return out


def kernel(**inputs):
    x = np.asarray(inputs["x"], np.float32)
    ei = np.asarray(inputs["edge_index"], np.int64)
    N, Fin = x.shape
    Nc = N // C
    Ncp = ((Nc + 127) // 128) * 128
    src, dst = ei[0], ei[1]

    deg = np.bincount(dst, minlength=N).astype(np.float32)
    dinv = np.where(deg > 0, 1.0 / np.sqrt(np.maximum(deg, 1.0)), 0.0).astype(np.float32)

    sched, gidx_all, dloc_all = _prep_graph(src, dst, N, Nc, Ncp)
    wts = _pack_weights(inputs)
    nc = _build_nc(Nc, Ncp, Fin, sched)

    G = Ncp // 128
    in_maps = []
    for c in range(C):
        xs = np.zeros((Ncp, Fin), np.float32)
        xs[:Nc] = x[c * Nc:(c + 1) * Nc]
        dl = np.zeros(Ncp, np.float32)
        dl[:Nc] = dinv[c * Nc:(c + 1) * Nc]
        dnm = dl.reshape(G, 128).T.copy()          # [128, G]
        m = dict(wts)
        m.update(
            x_sh=xs, gidx=gidx_all[c], dloc=dloc_all[c],
            dinv=np.ascontiguousarray(dnm),
        )
        in_maps.append(m)

    res = run_bass_kernel_spmd(nc, in_maps, list(range(C)), trace=TRACE[0])
    LAST["res"] = res

    out = np.zeros((N, 32), np.float32)
    for c in range(C):
        oc = np.asarray(res.results[c]["out"], np.float32)  # [128, G*32]
        oc = oc.reshape(128, G, 32).transpose(1, 0, 2).reshape(Ncp, 32)
        out[c * Nc:(c + 1) * Nc] = oc[:Nc]
    return out
